# revision 1
# baseline (speedup 1.0000x reference)
"""Trainium2 Bass kernel for nn_AttentionQKNorm (B=4, N=2048, C=1024, H=16, D=64).

Sharding (8 cores): core c -> batch b = c//2, head-group hg = c%2 (8 heads).
Tensor-parallel within a batch: each core computes qkv for its 8 heads,
per-head QK-LayerNorm, attention, and a partial projection
o_part @ w_proj[rows] -> [2048, 1024]. Host sums the two partials per batch
(row-parallel all-reduce done on host as part of unsharding).

Device layouts (all "transposed": feature dim on partitions):
  xT [C, N]: x[b].T                  (rhs / lhsT of qkv matmuls)
  qTf/kTf per head-pair [128, 2048]: rows = 2 heads x 64 d, QK-layernormed
  S^T chunks [128 k, 512 q] = K^T.T @ Q^T  (row-paired: both heads concurrent)
  P = exp(S^T * D^-0.5)  (softmax without max-subtraction: |S| <= ~6 here)
  PV: lhsT = [V_chunk | ones] (M=65) -> psum rows 0..63 = O^T, row 64 = denom
  proj: outT[col, tok] accumulated over the 4 head-pair row-chunks of w_proj

Matmul operand dtype: float32r (~1.7e-4 matmul rel err, full PE rate at N>=512).
"""

import numpy as np

H = 16
D = 64
B = 4
SEQ = 2048
C = 1024
NCORES = 8
NHP = 4  # head-pairs per core (8 heads)
EPS = 1e-6
SCALE = D**-0.5

_CACHE = {}


def _build_nc(reps=1):
    from contextlib import ExitStack

    import concourse.bacc as bacc
    import concourse.tile as tile
    import concourse.mybir as mybir

    dt = mybir.dt
    F32, F32R = dt.float32, dt.float32r
    AF = mybir.ActivationFunctionType
    ALU = mybir.AluOpType

    # Ln and Exp both live in act-table set "natural_log_exp_and_others",
    # but the table-load inserter picks the FIRST set containing each func
    # (natural_log for Ln, exp_and_others for Exp), causing a ~2.7us table
    # reload around every LayerNorm rstd. Hide Ln/Exp from the
    # earlier sets so both resolve to the shared set -> one load total.
    import concourse.hw_specs as hw_specs
    if not getattr(bacc, "_qknorm_act_tables_patched", False):
        _orig_get_tables = bacc.get_activation_tables

        def _patched_get_tables(arch):
            tabs = {k: set(v) for k, v in _orig_get_tables(arch).items()}
            af = mybir.ActivationFunctionType
            both = "natural_log_exp_and_others"
            if both in tabs and af.Exp in tabs[both] and af.Ln in tabs[both]:
                for name, funcs in tabs.items():
                    if name != both:
                        funcs.discard(af.Ln)
                        funcs.discard(af.Exp)
            return tabs

        bacc.get_activation_tables = _patched_get_tables
        bacc._qknorm_act_tables_patched = True

    nc = bacc.Bacc("TRN2", target_bir_lowering=False, debug=False,
                   num_devices=NCORES)

    def din(name, shape, dtype=F32R):
        return nc.dram_tensor(name, shape, dtype, kind="ExternalInput").ap()

    xT = din("xT", [C, SEQ])
    wq = din("wq", [C, 512])
    wk = din("wk", [C, 512])
    wv = din("wv", [C, 512])
    wp = din("wp", [512, C])
    bq = din("bq", [128, NHP], F32)
    bk = din("bk", [128, NHP], F32)
    bv = din("bv", [128, NHP], F32)
    bp = din("bp", [128, 8], F32)
    ones2 = din("ones2", [128, 2])
    ind2 = din("ind2", [2, 128])
    gqb = din("gq", [2, 128])
    gkb = din("gk", [2, 128])
    ones1 = din("ones1", [1, 128])
    epsc = din("epsc", [128, 1], F32)
    outT = nc.dram_tensor("outT", [C, SEQ], F32, kind="ExternalOutput").ap()
    # DRAM staging for per-head-pair attention outputs (reloaded for proj)
    oT_stage = nc.dram_tensor("oT_stage", [512, SEQ], F32R).ap()

    with tile.TileContext(nc) as tc, ExitStack() as ctx, \
            nc.allow_low_precision("float32r matmul operands by design"):
        ep = ctx.enter_context

        const_p = ep(tc.tile_pool(name="const", bufs=1))
        xt_p = ep(tc.tile_pool(name="xt", bufs=1))       # 8 tags x 8KB = 64K
        wv_p = ep(tc.tile_pool(name="wv", bufs=1))       # 8 tags x 2KB = 16K
        wqk_p = ep(tc.tile_pool(name="wqk", bufs=1))     # 2 tags x 4KB = 8K
        qk_p = ep(tc.tile_pool(name="qk", bufs=2))       # 2 tags x 2 x 8KB = 32K
        v_p = ep(tc.tile_pool(name="v", bufs=1))         # 4 tags x 8.125K = 32.5K
        ot_p = ep(tc.tile_pool(name="ot", bufs=1))       # 1 tag x 8KB = 8K
        ln_p = ep(tc.tile_pool(name="ln", bufs=3))       # qb: 3 x 2KB = 6K
        scr_p = ep(tc.tile_pool(name="scr", bufs=4))
        st_p = ep(tc.tile_pool(name="st", bufs=2))       # 3 tags x 2 x 2KB = 12K
        pch_p = ep(tc.tile_pool(name="pch", bufs=3))     # 1 tag x 2 x 4KB = 8K
        ps_big = ep(tc.tile_pool(name="psb", bufs=2, space="PSUM"))   # 4 banks
        ps_pv = ep(tc.tile_pool(name="pspv", bufs=2, space="PSUM"))   # 2 banks
        ps_sm = ep(tc.tile_pool(name="pssm", bufs=2, space="PSUM"))   # 2 banks

        for rep in range(reps):
            # ---- constant loads ----
            _const_dmas = []

            def load_const(ap_dram, shape, dtype):
                t = const_p.tile(shape, dtype, tag=ap_dram.tensor.name,
                                 name=f"r{rep}" + ap_dram.tensor.name)
                _const_dmas.append((t, ap_dram))
                return t

            def emit_consts():
                for t, ap_dram in _const_dmas:
                    nc.sync.dma_start(t[:], ap_dram)
                _const_dmas.clear()

            ones2_sb = load_const(ones2, [128, 2], F32R)
            ind2_sb = load_const(ind2, [2, 128], F32R)
            gq_sb = load_const(gqb, [2, 128], F32R)
            gk_sb = load_const(gkb, [2, 128], F32R)
            ones1_sb = load_const(ones1, [1, 128], F32R)
            bq_sb = load_const(bq, [128, NHP], F32)
            bk_sb = load_const(bk, [128, NHP], F32)
            bv_sb = load_const(bv, [128, NHP], F32)
            bp_sb = load_const(bp, [128, 8], F32)
            eps_sb = load_const(epsc, [128, 1], F32)

            xt_sb = []
            state = {}
            wv_loaded = []

            def load_wv():
                if wv_loaded:
                    return
                for kc in range(8):
                    t = wv_p.tile([128, 512], F32R, tag=f"wv{kc}",
                                  name=f"r{rep}wv{kc}")
                    nc.sync.dma_start(
                        t[:], wv[kc * 128:(kc + 1) * 128, :])
                    wv_loaded.append(t)

            def qkln_tasks(hp, bcast_pool=None):
                """q/k projection + QK-LayerNorm for head pair hp as a list
                of 8 per-tile closures (interleavable as PE filler work).
                bcast_pool: psum pool for the mean/rstd broadcast matmuls
                (ps_pv is free during the prologue; ps_sm otherwise)."""
                bcast_pool = bcast_pool or ps_sm
                bcast_tag = "pv" if bcast_pool is ps_pv else "sm"
                whs = {}
                for wname, wdram in (("k", wk), ("q", wq)):
                    wh = wqk_p.tile([128, 8 * 128], F32R, tag=f"w{wname}h",
                                    name=f"r{rep}w{wname}h{hp}")
                    nc.sync.dma_start(
                        wh[:].rearrange("p (kc c) -> p kc c", kc=8),
                        wdram[:, hp * 128:(hp + 1) * 128]
                        .rearrange("(kc p) c -> p kc c", p=128),
                    )
                    whs[wname] = wh
                    dst = qk_p.tile([128, SEQ], F32R, tag=f"{wname}T",
                                    name=f"r{rep}{wname}T{hp}")
                    state[hp, wname] = dst

                def tile_micro(wname, bias_sb, g_sb, n):
                    """One q/k LN tile as 4 micro-closures (~0.6-1.7us of PE
                    work each) so filler never delays an S-fill by much."""
                    wh = whs[wname]
                    dst = state[hp, wname]
                    c = {}

                    def t1():
                        ps = ps_big.tile([128, 1024], F32, tag="big",
                                         name=f"r{rep}psqk{hp}{wname}{n}")
                        for kc in range(4):
                            nc.tensor.matmul(
                                ps[:, 0:512],
                                lhsT=wh[:, kc * 128:(kc + 1) * 128],
                                rhs=xt_sb[kc][:, n * 512:(n + 1) * 512],
                                start=(kc == 0), stop=False,
                            )
                        c["ps"] = ps

                    def t2():
                        ps = c["ps"]
                        for kc in range(4, 8):
                            nc.tensor.matmul(
                                ps[:, 0:512],
                                lhsT=wh[:, kc * 128:(kc + 1) * 128],
                                rhs=xt_sb[kc][:, n * 512:(n + 1) * 512],
                                start=False, stop=(kc == 7),
                            )
                        qb_t = ln_p.tile([128, 512], F32R, tag="qb",
                                         name=f"r{rep}qb{hp}{wname}{n}")
                        nc.vector.tensor_scalar_add(
                            qb_t[:], ps[:, 0:512], bias_sb[:, hp:hp + 1])
                        q2 = scr_p.tile([128, 512], F32R, tag="scr",
                                        name=f"r{rep}q2{hp}{wname}{n}")
                        nc.vector.tensor_mul(q2[:], qb_t[:], qb_t[:])
                        c["qb"] = qb_t
                        c["q2"] = q2

                    def t3():
                        qb_t, q2 = c["qb"], c["q2"]
                        ps_sum = ps_sm.tile([128, 512], F32, tag="sm",
                                            name=f"r{rep}pssum{hp}{wname}{n}")
                        nc.tensor.matmul(ps_sum[0:2, :], lhsT=ones2_sb[:],
                                         rhs=qb_t[:], start=True, stop=True)
                        ps_sq = ps_sm.tile([128, 512], F32, tag="sm",
                                           name=f"r{rep}pssq{hp}{wname}{n}")
                        nc.tensor.matmul(ps_sq[0:2, :], lhsT=ones2_sb[:],
                                         rhs=q2[:], start=True, stop=True)
                        mu = st_p.tile([2, 512], F32R, tag="mu",
                                       name=f"r{rep}mu{hp}{wname}{n}")
                        nc.vector.tensor_scalar_mul(mu[:], ps_sum[0:2, :],
                                                    1.0 / D)
                        mu2 = st_p.tile([2, 512], F32, tag="lnstat", bufs=4,
                                        name=f"r{rep}mu2{hp}{wname}{n}")
                        nc.vector.tensor_mul(mu2[:], mu[:], mu[:])
                        var = st_p.tile([2, 512], F32, tag="lnstat", bufs=4,
                                        name=f"r{rep}var{hp}{wname}{n}")
                        nc.vector.scalar_tensor_tensor(
                            var[:], ps_sq[0:2, :], 1.0 / D, mu2[:],
                            op0=ALU.mult, op1=ALU.subtract)
                        lnv = st_p.tile([2, 512], F32, tag="lnstat", bufs=4,
                                        name=f"r{rep}lnv{hp}{wname}{n}")
                        nc.scalar.activation(lnv[:], var[:], AF.Ln,
                                             bias=eps_sb[0:2, 0:1])
                        rstd = st_p.tile([2, 512], F32R, tag="lnstat", bufs=4,
                                         name=f"r{rep}rstd{hp}{wname}{n}")
                        nc.scalar.activation(rstd[:], lnv[:], AF.Exp,
                                             scale=-0.5)
                        c["mu"] = mu
                        c["rstd"] = rstd

                    def t4():
                        qb_t, mu, rstd = c["qb"], c["mu"], c["rstd"]
                        ps_mb = bcast_pool.tile(
                            [128, 512], F32, tag=bcast_tag,
                            name=f"r{rep}psmb{hp}{wname}{n}")
                        nc.tensor.matmul(ps_mb[:], lhsT=ind2_sb[:], rhs=mu[:],
                                         start=True, stop=True)
                        ps_gr = bcast_pool.tile(
                            [128, 512], F32, tag=bcast_tag,
                            name=f"r{rep}psgr{hp}{wname}{n}")
                        nc.tensor.matmul(ps_gr[:], lhsT=g_sb[:], rhs=rstd[:],
                                         start=True, stop=True)
                        tdiff = scr_p.tile([128, 512], F32, tag="scr",
                                           name=f"r{rep}td{hp}{wname}{n}")
                        nc.vector.tensor_sub(tdiff[:], qb_t[:], ps_mb[:])
                        nc.vector.tensor_mul(
                            dst[:, n * 512:(n + 1) * 512], tdiff[:], ps_gr[:])

                    return [t1, t2, t3, t4]

                tasks = []
                for wname, bias_sb, g_sb in (("k", bk_sb, gk_sb),
                                             ("q", bq_sb, gq_sb)):
                    for n in range(4):
                        tasks.extend(tile_micro(wname, bias_sb, g_sb, n))
                return tasks

            def v_tasks():
                """V projection for ALL head pairs: 16 tok-chunk closures
                (N=512 matmuls; fp32r needs N>=256 for full PE rate).
                v_ext chunks: [v_h0 | 1 | v_h1 | 1] x16; ones-columns
                accumulate softmax denominators inside the PV matmul."""
                vxs = []
                for hp in range(NHP):
                    vx = v_p.tile([128, 16 * 130], F32R, tag=f"vx{hp}",
                                  name=f"r{rep}vx{hp}")
                    nc.vector.memset(vx[:].bitcast(F32), 1.0)
                    state[hp, "v"] = vx
                    vxs.append(vx)

                def chunk_task(t16):
                    ps = ps_big.tile([128, 1024], F32, tag="big",
                                     name=f"r{rep}psv{t16}")
                    for kc in range(8):
                        nc.tensor.matmul(
                            ps[:, 0:512],
                            lhsT=xt_sb[kc][:, t16 * 128:(t16 + 1) * 128],
                            rhs=wv_loaded[kc][:],
                            start=(kc == 0), stop=(kc == 7),
                        )
                    for hp in range(NHP):
                        dst = vxs[hp][:, t16 * 130:(t16 + 1) * 130]
                        nc.vector.tensor_copy(
                            dst[:, 0:130]
                            .rearrange("p (two s) -> p two s", s=65)
                            [:, :, 0:64],
                            ps[:, hp * 128:(hp + 1) * 128]
                            .rearrange("p (two s) -> p two s", s=64),
                        )

                return [lambda t=t16: chunk_task(t) for t16 in range(16)]

            # ---- flattened attention pipeline over all head pairs ----
            pvs_all = {}
            ps_tiles = {}
            oTs = {}

            def emit_s(hp, qb_i, g, h):
                qT, kT = state[hp, "q"], state[hp, "k"]
                qs = slice(qb_i * 512, (qb_i + 1) * 512)
                ps_s = ps_big.tile([128, 1024], F32, tag="big",
                                   name=f"r{rep}pss{hp}{qb_i}{g}{h}")
                for j in range(2):
                    kc = 2 * g + j
                    nc.tensor.matmul(
                        ps_s[:, j * 512:(j + 1) * 512],
                        lhsT=kT[slice(64 * h, 64 * h + 64),
                                kc * 128:(kc + 1) * 128],
                        rhs=qT[slice(64 * h, 64 * h + 64), qs],
                        start=True, stop=True,
                    )
                ps_tiles[hp, qb_i, g, h] = ps_s

            def emit_exp_pv(hp, qb_i, g, h):
                ps_s = ps_tiles.pop((hp, qb_i, g, h))
                vx = state[hp, "v"]
                pg = pch_p.tile([128, 1024], F32R, tag="pg",
                                name=f"r{rep}pg{hp}{qb_i}{g}{h}")
                nc.scalar.activation(pg[:], ps_s[:], AF.Exp, scale=SCALE)
                pvs = pvs_all[hp, qb_i]
                for j in range(2):
                    kc = 2 * g + j
                    nc.tensor.matmul(
                        pvs[h][0:65, :],
                        lhsT=vx[:, kc * 130 + 65 * h:kc * 130 + 65 * h + 65],
                        rhs=pg[:, j * 512:(j + 1) * 512],
                        start=(kc == 0), stop=(kc == 15),
                    )

            def emit_divide(hp, qb_i):
                """Evacuate PV psums (incl. denominator row) immediately to
                free the banks; recip/broadcast/divide then run from SBUF."""
                if hp not in oTs:
                    # oT(1)/oT(2) live in retired v_ext slots (free after
                    # their head pair's attention) -> no DRAM stage+reload
                    if hp in (1, 2):
                        oTs[hp] = v_p.tile([128, SEQ], F32R,
                                           tag=f"vx{hp - 1}",
                                           name=f"r{rep}ot{hp}")
                    else:
                        oTs[hp] = ot_p.tile([128, SEQ], F32R, tag="ot",
                                            name=f"r{rep}ot{hp}")
                oT = oTs[hp]
                qs = slice(qb_i * 512, (qb_i + 1) * 512)
                pvs = pvs_all.pop((hp, qb_i))
                ocps = []
                for h in range(2):
                    ocp = scr_p.tile([128, 512], F32, tag="scr",
                                     name=f"r{rep}ocp{hp}{qb_i}{h}")
                    nc.vector.tensor_copy(ocp[0:65, :], pvs[h][0:65, :])
                    ocps.append(ocp)
                for h in range(2):
                    ocp = ocps[h]
                    # wv slots are dead once V is built (end of hp0/qb0)
                    rh = wv_p.tile([1, 512], F32R, tag=f"wv{h}",
                                   name=f"r{rep}rh{hp}{qb_i}{h}")
                    nc.vector.reciprocal(rh[:], ocp[64:65, :])
                    ps_rb = ps_sm.tile([128, 512], F32, tag="sm",
                                       name=f"r{rep}psrb{hp}{qb_i}{h}")
                    nc.tensor.matmul(
                        ps_rb[0:64, :],
                        lhsT=ones1_sb[0:1, 0:64], rhs=rh[:],
                        start=True, stop=True,
                    )
                    dstv = oT[64 * h:64 * h + 64, qs]
                    nc.vector.tensor_mul(dstv, ocp[0:64, :], ps_rb[0:64, :])
                    nc.vector.tensor_scalar_add(
                        dstv, dstv, bv_sb[64 * h:64 * h + 64, hp:hp + 1])

            # ---- prologue: head pair 0's q/k/LN before any attention.
            # qkln_tasks(0) emits the small w-slice DMAs first so the first
            # matmuls aren't stuck behind the full 8MiB x^T load. ----
            t0_tasks = qkln_tasks(0, bcast_pool=ps_pv)
            for kc in range(8):
                t = xt_p.tile([128, SEQ], F32R, tag=f"xt{kc}",
                              name=f"r{rep}xt{kc}")
                xt_sb.append(t)
            for n in range(4):
                for kc in range(8):
                    nc.sync.dma_start(
                        xt_sb[kc][:, n * 512:(n + 1) * 512],
                        xT[kc * 128:(kc + 1) * 128, n * 512:(n + 1) * 512])
                if n == 0:
                    emit_consts()
            for t in t0_tasks:
                t()
            load_wv()
            vts = v_tasks()  # V matmuls run inline during attention(0, qb0)

            # ---- main pipeline: 256 attention steps with qkv/LN and V
            # interleaved as PE filler in the exp latency shadow ----
            steps = [(hp, qb_i, g, h)
                     for hp in range(NHP) for qb_i in range(4)
                     for g in range(8) for h in range(2)]
            fill = {}
            for i, tv in enumerate(vts):
                fill.setdefault(i, []).append(tv)  # V during hp0/qb0

            def place(hp, qb_i, tasks):
                for j, t in enumerate(tasks):
                    local = 1 + (j * 14) // len(tasks) if len(tasks) > 2 \
                        else (6, 12)[len(tasks) - 1 - j if False else j]
                    fill.setdefault(hp * 64 + qb_i * 16 + local, []).append(t)

            for hp in range(NHP - 1):
                nxt = qkln_tasks(hp + 1)
                if hp == 0:
                    for i in range(3):
                        k = len(nxt)
                        place(0, 1 + i, nxt[(i * k) // 3:((i + 1) * k) // 3])
                else:
                    for i in range(4):
                        k = len(nxt)
                        place(hp, i, nxt[(i * k) // 4:((i + 1) * k) // 4])

            for i, st in enumerate(steps):
                hp, qb_i, g, h = st
                if (g, h) == (0, 0):
                    pvs_all[hp, qb_i] = [
                        ps_pv.tile([128, 512], F32, tag="pv",
                                   name=f"r{rep}pv{hp}{qb_i}{_h}")
                        for _h in range(2)]
                emit_s(*st)
                for t in fill.get(i, ()):
                    t()
                if i >= 1:
                    emit_exp_pv(*steps[i - 1])
                if i >= 1 and steps[i - 1][2:] == (7, 1):
                    php, pqb = steps[i - 1][:2]
                    emit_divide(php, pqb)
                    if pqb == 3 and php == 0:
                        nc.sync.dma_start(
                            oT_stage[0:128, :], oTs[0][:])
            emit_exp_pv(*steps[-1])
            emit_divide(NHP - 1, 3)
            state["oT3"] = oTs[NHP - 1]

            # ---- projection: outT[col, tok] = wp.T @ oT (+ b_proj) ----
            # head pair 3's output is used directly from SBUF; 0-2 reload
            # from DRAM into slots retired by qT/kT/x^T (DMA overlaps the
            # tail of attention).
            otr0 = qk_p.tile([128, SEQ], F32R, tag="qT",
                             name=f"r{rep}otr0")
            nc.sync.dma_start(otr0[:], oT_stage[0:128, :])
            otr = [otr0, oTs[1], oTs[2], state["oT3"]]
            rr = [(ps_big, "big", 1024), (ps_pv, "pv", 512),
                  (ps_sm, "sm", 512)]
            for m in range(8):
                wpm = xt_p.tile([128, 4 * 128], F32R, tag=f"xt{3 + m % 4}",
                                name=f"r{rep}wpm{m}")
                nc.sync.dma_start(
                    wpm[:].rearrange("p (hp c) -> p hp c", hp=4),
                    wp[:, m * 128:(m + 1) * 128]
                    .rearrange("(hp p) c -> p hp c", p=128),
                )
                for n in range(4):
                    pool, tag, width = rr[(m * 4 + n) % 3]
                    ps = pool.tile([128, width], F32, tag=tag,
                                   name=f"r{rep}pspr{m}{n}")
                    for hp in range(NHP):
                        nc.tensor.matmul(
                            ps[:, 0:512],
                            lhsT=wpm[:, hp * 128:(hp + 1) * 128],
                            rhs=otr[hp][:, n * 512:(n + 1) * 512],
                            start=(hp == 0), stop=(hp == NHP - 1),
                        )
                    so = scr_p.tile([128, 512], F32, tag="scr",
                                    name=f"r{rep}so{m}{n}")
                    nc.scalar.activation(so[:], ps[:, 0:512], AF.Identity,
                                         bias=bp_sb[:, m:m + 1])
                    nc.sync.dma_start(
                        outT[m * 128:(m + 1) * 128,
                             n * 512:(n + 1) * 512], so[:])

    nc.compile()
    return nc


def make_in_maps(x, w_qkv, b_qkv, g_q, g_k, w_proj, b_proj):
    """Host-side sharding: per-core input dict."""
    f32 = np.float32
    x = np.ascontiguousarray(x, dtype=f32)
    w_qkv = np.asarray(w_qkv, dtype=f32)
    b_qkv = np.asarray(b_qkv, dtype=f32)
    g_q = np.asarray(g_q, dtype=f32)
    g_k = np.asarray(g_k, dtype=f32)
    w_proj = np.asarray(w_proj, dtype=f32)
    b_proj = np.asarray(b_proj, dtype=f32)

    ones2 = np.zeros((128, 2), f32)
    ones2[0:64, 0] = 1.0
    ones2[64:128, 1] = 1.0
    ind2 = np.ascontiguousarray(ones2.T)
    gqb = np.zeros((2, 128), f32)
    gqb[0, 0:64] = g_q
    gqb[1, 64:128] = g_q
    gkb = np.zeros((2, 128), f32)
    gkb[0, 0:64] = g_k
    gkb[1, 64:128] = g_k
    ones1 = np.ones((1, 128), f32)

    in_maps = []
    for c in range(NCORES):
        b = c // 2
        hg = c % 2
        cs = slice(hg * 512, (hg + 1) * 512)
        in_maps.append({
            "xT": np.ascontiguousarray(x[b].T),
            "wq": np.ascontiguousarray(w_qkv[:, hg * 512:(hg + 1) * 512]),
            "wk": np.ascontiguousarray(w_qkv[:, C + hg * 512:C + (hg + 1) * 512]),
            "wv": np.ascontiguousarray(
                w_qkv[:, 2 * C + hg * 512:2 * C + (hg + 1) * 512]),
            "wp": np.ascontiguousarray(w_proj[hg * 512:(hg + 1) * 512, :]),
            "bq": np.ascontiguousarray(b_qkv[cs].reshape(NHP, 128).T),
            "bk": np.ascontiguousarray(b_qkv[C:][cs].reshape(NHP, 128).T),
            "bv": np.ascontiguousarray(b_qkv[2 * C:][cs].reshape(NHP, 128).T),
            "bp": np.ascontiguousarray(
                (b_proj if hg == 0 else np.zeros_like(b_proj))
                .reshape(8, 128).T),
            "ones2": ones2,
            "ind2": ind2,
            "gq": gqb,
            "gk": gkb,
            "ones1": ones1,
            "epsc": np.full((128, 1), EPS, f32),
        })
    return in_maps


def unshard(partials):
    """partials: list of 8 outT arrays [C, SEQ] -> full [B, SEQ, C]."""
    out = np.empty((B, SEQ, C), np.float32)
    for b in range(B):
        out[b] = (partials[2 * b] + partials[2 * b + 1]).T
    return out


def kernel(x, w_qkv, b_qkv, g_q, g_k, w_proj, b_proj):
    from concourse.bass_utils import run_bass_kernel_spmd

    if "nc" not in _CACHE:
        _CACHE["nc"] = _build_nc()
    nc = _CACHE["nc"]
    in_maps = make_in_maps(x, w_qkv, b_qkv, g_q, g_k, w_proj, b_proj)
    res = run_bass_kernel_spmd(nc, in_maps, list(range(NCORES)))
    return unshard([res.results[c]["outT"] for c in range(NCORES)])

